# revision 6
# baseline (speedup 1.0000x reference)
"""Trainium2 Bass kernel for nn_Network_47502338294626 (gnn_message_passing).

Strategy: pure data parallel over the batch dim across 8 NeuronCores.
The whole per-row network folds into ~12 fused feature-major matmul stages
(float32r, N=512 batch columns) plus a batch-major "terminal" phase that
emits all 5 outputs through one PSUM bank per 128-row chunk, giving
contiguous row-major DMA writes with no transposes of the big outputs.
All biases ride on "ones" rows carried inside the state tiles, so every
PSUM evacuation is a single activation/copy op.
"""

import numpy as np
from contextlib import ExitStack

B_TOTAL = 131072
N_CORES = 8
ROWS_PER_CORE = B_TOTAL // N_CORES
NB = 512                     # batch columns per feature-major block

# terminal TP column layout (366 columns in one PSUM bank)
TP_W = 366
TC_L1, TC_L4, TC_O6, TC_DEC, TC_D4, TC_HD = 0, 130, 180, 182, 312, 362

F32R_WEIGHTS = [
    "WZ1", "WD1A", "WD1B", "WO2A", "WO2X", "WO3", "WO3L", "WO4O", "WO4X",
    "WY4", "WO5", "WZ6O", "WZ6X", "WT_X",
]

WEIGHT_SHAPES = {
    "WZ1": (14, 121), "WD1A": (121, 128), "WD1B": (121, 2),
    "WO2A": (121, 71), "WO2X": (14, 71), "WO3": (71, 101),
    "WO3L": (71, 55),
    "WO4O": (101, 51), "WO4X": (14, 51), "WY4": (51, 55),
    "WO5": (51, 101), "WZ6O": (101, 2), "WZ6X": (14, 2),
    "WT_X": (14, 312), "WT_O6": (2, 2),
    "WT_A": (128, 130), "WT_B": (2, 130), "WT_C1": (101, 50),
    "WT_D1": (55, 50), "WT_D2": (55, 4),
}


def round_f32r(a):
    """Round fp32 to the PE's FP32R format (8-bit exp, 11-bit mantissa, RNE)."""
    u = np.asarray(a, np.float32).view(np.uint32).astype(np.uint64)
    lsb = (u >> 12) & 1
    u = (u + 0x7FF + lsb) & ~np.uint64(0xFFF)
    return u.astype(np.uint32).view(np.float32)


# --------------------------------------------------------------------------
# fold-matrix construction (pure numpy; math validated vs the jax reference)
# --------------------------------------------------------------------------
def build_folds(inp):
    f32 = np.float32
    g = {k: np.asarray(v, dtype=np.float64) for k, v in inp.items()}
    w0, b0 = g["w0"], g["b0"]
    E1, E2, E4, E6 = g["E1"], g["E2"], g["E4"], g["E6"]
    W1, b1 = g["W1"], g["b1"]
    W2, b2 = g["W2"], g["b2"]
    W3, b3 = g["W3"], g["b3"]
    W4, b4 = g["W4"], g["b4"]
    W5, b5 = g["W5"], g["b5"]
    W6, b6 = g["W6"], g["b6"]
    decW1, decb1, decW2, decb2 = g["decW1"], g["decb1"], g["decW2"], g["decb2"]
    dec4W1, dec4b1, dec4W2, dec4b2 = g["dec4W1"], g["dec4b1"], g["dec4W2"], g["dec4b2"]
    od2W, od2b = g["od2W"], g["od2b"]
    od40W, od40b = g["od40W"], g["od40b"]
    od41W, od41b = g["od41W"], g["od41b"]
    od42W, od42b = g["od42W"], g["od42b"]

    lrelu = lambda v: np.where(v >= 0, v, 0.01 * v)

    V1 = W1.sum(-1)
    R2 = W2.sum(-1)
    R4 = W4.sum(-1)
    R6 = W6.sum(-1)[:, 0]
    W2m = W2.transpose(2, 0, 1).reshape(10, 70)
    W3m = W3.transpose(2, 0, 1).reshape(10, 100)
    W4m = W4.transpose(2, 0, 1).reshape(10, 50)
    W5m = W5.transpose(2, 0, 1).reshape(10, 100)
    W6m = W6[:, 0, :].T

    var1 = [n for n in range(13) if np.abs(E1[:, n]).sum() > 0]
    const1 = [n for n in range(13) if n not in var1]
    nv = len(var1)
    assert nv == 12, f"unexpected E1 sparsity: {var1}"

    A1x_full = np.zeros((13, 130))
    c1_full = np.zeros(130)
    b0E1 = b0 @ E1
    for n in range(13):
        for o in range(10):
            A1x_full[:, 10 * n + o] = w0 * E1[:, n] * V1[n, o]
            c1_full[10 * n + o] = b0E1[n] * V1[n, o] + b1[n, o]
    var_rows = [10 * n + o for n in var1 for o in range(10)]
    const_rows = [10 * n + o for n in const1 for o in range(10)]
    A1x = A1x_full[:, var_rows]
    c1 = c1_full[var_rows]
    out1_const = lrelu(c1_full[const_rows])

    D1 = decW1.T
    D1v = D1[var_rows, :]
    bias_y1 = decb1 + out1_const @ D1[const_rows, :]

    S1v = np.zeros((120, 10))
    for vi in range(nv):
        for o in range(10):
            S1v[10 * vi + o, o] = 1.0
    sc1 = np.zeros(10)
    for ci in range(len(const1)):
        for o in range(10):
            sc1[o] += out1_const[10 * ci + o]

    b0E2 = b0 @ E2
    A2x = np.zeros((13, 70))
    c2 = np.zeros(70)
    for n in range(7):
        for o in range(10):
            A2x[:, 10 * n + o] = w0 * E2[:, n] * R2[n, o]
            c2[10 * n + o] = b0E2[n] * R2[n, o] + b2[n, o]
    c2 = c2 + sc1 @ W2m
    A2o = S1v @ W2m

    S2 = np.zeros((70, 10))
    for n in range(7):
        for o in range(10):
            S2[10 * n + o, o] = 1.0
    A3 = S2 @ W3m
    c3 = b3.reshape(100).copy()
    A3l2o = np.zeros((70, 1))
    A3l2o[30:40, 0] = od2W[0, :]

    S3 = np.zeros((100, 10))
    for n in range(10):
        for o in range(10):
            S3[10 * n + o, o] = 1.0
    b0E4 = b0 @ E4
    A4x = np.zeros((13, 50))
    c4 = np.zeros(50)
    W4inx = np.zeros((13, 50))
    c_l4 = np.zeros(50)
    for n in range(5):
        for o in range(10):
            A4x[:, 10 * n + o] = w0 * E4[:, n] * R4[n, o]
            c4[10 * n + o] = b0E4[n] * R4[n, o] + b4[n, o]
            W4inx[:, 10 * n + o] = w0 * E4[:, n]
            c_l4[10 * n + o] = b0E4[n]
    A4o = S3 @ W4m
    T4 = np.zeros((10, 50))
    for n in range(5):
        for i in range(10):
            T4[i, 10 * n + i] = 1.0
    W4ino = S3 @ T4

    B4 = dec4W1.T
    d4 = dec4b1
    H4 = np.zeros((50, 3))
    H4[0:10, 0] = od40W[0, :]
    H4[10:20, 1] = od41W[0, :]
    H4[20:30, 2] = od42W[0, :]
    hb = np.array([od40b[0], od41b[0], od42b[0]])
    S4 = np.zeros((50, 10))
    for n in range(5):
        for o in range(10):
            S4[10 * n + o, o] = 1.0
    A5 = S4 @ W5m
    c5 = b5.reshape(100).copy()

    S5 = np.zeros((100, 10))
    for n in range(10):
        for o in range(10):
            S5[10 * n + o, o] = 1.0
    b0E6 = b0 @ E6
    A6x = np.zeros((13, 2))
    c6 = np.zeros(2)
    for n in range(2):
        A6x[:, n] = w0 * E6[:, n] * R6[n]
        c6[n] = b0E6[n] * R6[n] + b6[n, 0]
    A6o = S5 @ W6m

    W1inx = np.zeros((13, 130))
    c_l1 = np.zeros(130)
    for n in range(13):
        for i in range(10):
            W1inx[:, 10 * n + i] = w0 * E1[:, n]
            c_l1[10 * n + i] = b0E1[n]

    D2 = decW2.T
    D4 = dec4W2.T

    F = {}

    def mk(name, arr):
        a = np.ascontiguousarray(np.asarray(arr, dtype=f32))
        assert a.shape == WEIGHT_SHAPES[name], (name, a.shape)
        F[name] = a

    WZ1 = np.zeros((14, 121))
    WZ1[0:13, 0:120] = A1x
    WZ1[13, 0:120] = c1
    WZ1[13, 120] = 1.0
    mk("WZ1", WZ1)

    WD1A = np.zeros((121, 128))
    WD1A[0:120, :] = D1v[:, 0:128]
    WD1A[120, :] = bias_y1[0:128]
    mk("WD1A", WD1A)

    WD1B = np.zeros((121, 2))
    WD1B[0:120, :] = D1v[:, 128:130]
    WD1B[120, :] = bias_y1[128:130]
    mk("WD1B", WD1B)

    WO2A = np.zeros((121, 71))
    WO2A[0:120, 0:70] = A2o
    mk("WO2A", WO2A)

    WO2X = np.zeros((14, 71))
    WO2X[0:13, 0:70] = A2x
    WO2X[13, 0:70] = c2
    WO2X[13, 70] = 1.0
    mk("WO2X", WO2X)

    WO3 = np.zeros((71, 101))
    WO3[0:70, 0:100] = A3
    WO3[70, 0:100] = c3
    WO3[70, 100] = 1.0
    mk("WO3", WO3)

    WO3L = np.zeros((71, 55))
    WO3L[0:70, 53:54] = A3l2o
    WO3L[70, 53] = od2b[0]
    mk("WO3L", WO3L)

    WO4O = np.zeros((101, 51))
    WO4O[0:100, 0:50] = A4o
    WO4O[100, 0:50] = c4
    WO4O[100, 50] = 1.0
    mk("WO4O", WO4O)

    WO4X = np.zeros((14, 51))
    WO4X[0:13, 0:50] = A4x
    mk("WO4X", WO4X)

    WY4 = np.zeros((51, 55))
    WY4[0:50, 0:50] = B4
    WY4[0:50, 50:53] = H4
    WY4[50, 0:50] = d4
    WY4[50, 50:53] = hb
    WY4[50, 54] = 1.0
    mk("WY4", WY4)

    WO5 = np.zeros((51, 101))
    WO5[0:50, 0:100] = A5
    WO5[50, 0:100] = c5
    WO5[50, 100] = 1.0
    mk("WO5", WO5)

    WZ6O = np.zeros((101, 2))
    WZ6O[0:100, :] = A6o
    WZ6O[100, :] = c6
    mk("WZ6O", WZ6O)

    WZ6X = np.zeros((14, 2))
    WZ6X[0:13, :] = A6x
    mk("WZ6X", WZ6X)

    # terminal: single wide rhs for the XT stationary (l1in | l4in-x | out6 | dec-bias)
    WT_X = np.zeros((14, 312))
    WT_X[0:13, TC_L1:TC_L1 + 130] = W1inx
    WT_X[13, TC_L1:TC_L1 + 130] = c_l1
    WT_X[0:13, TC_L4:TC_L4 + 50] = W4inx
    WT_X[13, TC_L4:TC_L4 + 50] = c_l4
    WT_X[13, TC_DEC:TC_DEC + 130] = decb2
    mk("WT_O6", np.eye(2))
    mk("WT_X", WT_X)

    mk("WT_A", D2[0:128, :])
    mk("WT_B", D2[128:130, :])

    WT_C1 = np.zeros((101, 50))
    WT_C1[0:100, :] = W4ino
    mk("WT_C1", WT_C1)

    WT_D1 = np.zeros((55, 50))
    WT_D1[0:50, :] = D4
    WT_D1[54, :] = dec4b2
    mk("WT_D1", WT_D1)

    WT_D2 = np.zeros((55, 4))
    WT_D2[53, 0] = 1.0
    WT_D2[50, 1] = 1.0
    WT_D2[51, 2] = 1.0
    WT_D2[52, 3] = 1.0
    mk("WT_D2", WT_D2)

    for name in F32R_WEIGHTS:
        F[name] = round_f32r(F[name])
    return F


# --------------------------------------------------------------------------
# bass kernel
# --------------------------------------------------------------------------
def build_kernel(rows, use_prelu=True):
    import concourse.bacc as bacc
    import concourse.tile as tile
    from concourse import mybir

    f32 = mybir.dt.float32
    f32r = mybir.dt.float32r
    nblk = rows // NB
    assert rows % NB == 0

    nc = bacc.Bacc("TRN2", target_bir_lowering=False, debug=False)

    x_d = nc.dram_tensor("x", [rows, 13], f32, kind="ExternalInput")
    ident_d = nc.dram_tensor("ident", [128, 128], f32, kind="ExternalInput")
    ones_d = nc.dram_tensor("ones", [1, NB], f32r, kind="ExternalInput")
    w_d = {
        name: nc.dram_tensor(
            name, list(shape), f32r if name in F32R_WEIGHTS else f32,
            kind="ExternalInput")
        for name, shape in WEIGHT_SHAPES.items()
    }
    out_d = nc.dram_tensor("out", [rows, 6], f32, kind="ExternalOutput")
    dec_d = nc.dram_tensor("decoded", [rows, 130], f32, kind="ExternalOutput")
    l1_d = nc.dram_tensor("l1in", [rows, 130], f32, kind="ExternalOutput")
    d4_d = nc.dram_tensor("dec4", [rows, 50], f32, kind="ExternalOutput")
    l4_d = nc.dram_tensor("l4in", [rows, 50], f32, kind="ExternalOutput")

    AF = mybir.ActivationFunctionType
    ALU = mybir.AluOpType

    with tile.TileContext(nc) as tc, ExitStack() as ctx:
        wpool = ctx.enter_context(tc.tile_pool(name="weights", bufs=1))
        sb = ctx.enter_context(tc.tile_pool(name="state", bufs=3))
        tout_pool = ctx.enter_context(tc.tile_pool(name="tout", bufs=2))
        xb_pool = ctx.enter_context(tc.tile_pool(name="xb", bufs=3))
        ps = ctx.enter_context(tc.tile_pool(name="ps", bufs=4, space="PSUM"))
        tps = ctx.enter_context(tc.tile_pool(name="tps", bufs=4, space="PSUM"))

        W = {}
        for name, shape in WEIGHT_SHAPES.items():
            t = wpool.tile(list(shape), f32r if name in F32R_WEIGHTS else f32,
                           tag=name)
            nc.sync.dma_start(t[:], w_d[name][:])
            W[name] = t
        ident = wpool.tile([128, 128], f32, tag="ident")
        nc.sync.dma_start(ident[:], ident_d[:])

        def act(out_ap, in_ap):
            if use_prelu:
                nc.scalar.activation(out_ap, in_ap, AF.Lrelu, bias=0.0,
                                     scale=1.0, alpha=0.01)
            else:
                nc.scalar.activation(out_ap, in_ap, AF.Relu, bias=0.0,
                                     scale=1.0)

        for b in range(nblk):
            r0 = b * NB
            # ---- load x block, transpose to feature-major -----------------
            xb = xb_pool.tile([128, 52], f32, tag="xb")
            nc.sync.dma_start(
                xb[:].rearrange("p (c f) -> p c f", c=4),
                x_d[r0:r0 + NB, :].rearrange("(c p) f -> p c f", p=128),
            )
            pX = ps.tile([13, NB], f32, tag="ps")
            for c in range(4):
                nc.tensor.transpose(
                    pX[0:13, c * 128:(c + 1) * 128],
                    xb[:, c * 13:(c + 1) * 13],
                    ident[:],
                )
            XT = sb.tile([14, NB], f32r, tag="XT")
            nc.vector.tensor_copy(XT[0:13, :], pX[0:13, :])
            nc.sync.dma_start(XT[13:14, :], ones_d[:])

            # ---- feature-major chain (float32r matmuls, N=512) ------------
            p1 = ps.tile([121, NB], f32, tag="ps")
            nc.tensor.matmul(p1[:], W["WZ1"][:], XT[0:14, :], start=True, stop=True)
            OUT1 = sb.tile([121, NB], f32r, tag="OUT1")
            act(OUT1[:], p1[:])

            pA = ps.tile([128, NB], f32, tag="ps")
            nc.tensor.matmul(pA[:], W["WD1A"][:], OUT1[:], start=True, stop=True)
            Y1A = sb.tile([128, NB], f32r, tag="Y1A")
            nc.vector.tensor_scalar(Y1A[:], pA[:], 0.0, None, ALU.max)

            pB = ps.tile([2, NB], f32, tag="ps")
            nc.tensor.matmul(pB[:], W["WD1B"][:], OUT1[:], start=True, stop=True)
            YB2 = sb.tile([2, NB], f32r, tag="YB2")
            nc.vector.tensor_scalar(YB2[:], pB[:], 0.0, None, ALU.max)

            pC = ps.tile([71, NB], f32, tag="ps")
            nc.tensor.matmul(pC[:], W["WO2A"][:], OUT1[:], start=True, stop=False)
            nc.tensor.matmul(pC[:], W["WO2X"][:], XT[0:14, :], start=False, stop=True)
            OUT2 = sb.tile([71, NB], f32r, tag="OUT2")
            act(OUT2[:], pC[:])

            pD = ps.tile([101, NB], f32, tag="ps")
            nc.tensor.matmul(pD[:], W["WO3"][:], OUT2[:], start=True, stop=True)
            OT3 = sb.tile([101, NB], f32r, tag="OT3")
            act(OT3[:], pD[:])

            pE = ps.tile([51, NB], f32, tag="ps")
            nc.tensor.matmul(pE[:], W["WO4O"][:], OT3[:], start=True, stop=False)
            nc.tensor.matmul(pE[:], W["WO4X"][:], XT[0:14, :], start=False, stop=True)
            OUT4 = sb.tile([51, NB], f32r, tag="OUT4")
            act(OUT4[:], pE[:])

            pF = ps.tile([55, NB], f32, tag="ps")
            nc.tensor.matmul(pF[:], W["WY4"][:], OUT4[:], start=True, stop=False)
            nc.tensor.matmul(pF[:], W["WO3L"][:], OUT2[:], start=False, stop=True)
            Y4H = sb.tile([55, NB], f32r, tag="Y4H")
            nc.vector.tensor_scalar(Y4H[:], pF[:], 0.0, None, ALU.max)

            pG = ps.tile([101, NB], f32, tag="ps")
            nc.tensor.matmul(pG[:], W["WO5"][:], OUT4[:], start=True, stop=True)
            OUT5 = sb.tile([101, NB], f32r, tag="OUT5")
            act(OUT5[:], pG[:])

            pH = ps.tile([2, NB], f32, tag="ps")
            nc.tensor.matmul(pH[:], W["WZ6O"][:], OUT5[:], start=True, stop=False)
            nc.tensor.matmul(pH[:], W["WZ6X"][:], XT[0:14, :], start=False, stop=True)
            OUT6 = sb.tile([2, NB], f32r, tag="OUT6")
            act(OUT6[:], pH[:])

            # ---- terminal batch-major phase -------------------------------
            TOUT = tout_pool.tile([128, 4 * TP_W], f32, tag="TOUT")
            for c in range(4):
                cs = slice(c * 128, (c + 1) * 128)
                tp = tps.tile([128, TP_W], f32, tag="tp")
                nc.tensor.matmul(tp[:, 0:312], XT[:, cs], W["WT_X"][:],
                                 start=True, stop=False)
                nc.tensor.matmul(tp[:, TC_O6:TC_O6 + 2],
                                 OUT6[:, cs].bitcast(f32),
                                 W["WT_O6"][:].bitcast(f32),
                                 start=False, stop=False)
                nc.tensor.matmul(tp[:, TC_DEC:TC_DEC + 130],
                                 Y1A[:, cs].bitcast(f32), W["WT_A"][:],
                                 start=False, stop=False)
                nc.tensor.matmul(tp[:, TC_DEC:TC_DEC + 130],
                                 YB2[:, cs].bitcast(f32), W["WT_B"][:],
                                 start=False, stop=False)
                nc.tensor.matmul(tp[:, TC_L4:TC_L4 + 50],
                                 OT3[:, cs].bitcast(f32), W["WT_C1"][:],
                                 start=False, stop=False)
                nc.tensor.matmul(tp[:, TC_D4:TC_D4 + 50],
                                 Y4H[:, cs].bitcast(f32), W["WT_D1"][:],
                                 start=False, stop=False)
                nc.tensor.matmul(tp[:, TC_HD:TC_HD + 4],
                                 Y4H[:, cs].bitcast(f32), W["WT_D2"][:],
                                 start=False, stop=True)
                dst = TOUT[:, c * TP_W:(c + 1) * TP_W]
                if c % 2 == 0:
                    nc.vector.tensor_copy(dst, tp[:])
                else:
                    nc.scalar.copy(dst, tp[:])

            # ---- output DMAs ---------------------------------------------
            tv = TOUT[:].rearrange("p (c f) -> p c f", c=4)
            for dram, cols, c0, w in (
                (dec_d, slice(0, 130), TC_DEC, 130),
                (l1_d, slice(0, 130), TC_L1, 130),
                (l4_d, slice(0, 50), TC_L4, 50),
                (d4_d, slice(0, 50), TC_D4, 50),
                (out_d, slice(0, 4), TC_HD, 4),
                (out_d, slice(4, 6), TC_O6, 2),
            ):
                nc.sync.dma_start(
                    dram[r0:r0 + NB, cols].rearrange("(c p) f -> p c f", p=128),
                    tv[:, :, c0:c0 + w],
                )

    nc.compile()
    return nc


_KERNEL_CACHE = {}


def _get_kernel(rows, use_prelu=True):
    key = (rows, use_prelu)
    if key not in _KERNEL_CACHE:
        _KERNEL_CACHE[key] = build_kernel(rows, use_prelu)
    return _KERNEL_CACHE[key]


def run_on_cores(x_full, folds, n_cores=N_CORES, trace=False, use_prelu=True):
    from concourse.bass_utils import run_bass_kernel_spmd

    x_full = np.ascontiguousarray(np.asarray(x_full, dtype=np.float32))
    total = x_full.shape[0]
    rows = total // n_cores
    assert rows * n_cores == total
    nc = _get_kernel(rows, use_prelu)

    base = {name: folds[name] for name in WEIGHT_SHAPES}
    base["ident"] = np.eye(128, dtype=np.float32)
    base["ones"] = np.ones((1, NB), dtype=np.float32)
    in_maps = []
    for i in range(n_cores):
        m = dict(base)
        m["x"] = x_full[i * rows:(i + 1) * rows]
        in_maps.append(m)

    res = run_bass_kernel_spmd(nc, in_maps, list(range(n_cores)), trace=trace)
    outs = []
    for name in ("out", "decoded", "l1in", "dec4", "l4in"):
        outs.append(np.concatenate([res.results[i][name] for i in range(n_cores)],
                                   axis=0))
    return outs, res


def kernel(**inputs):
    folds = build_folds(inputs)
    x = np.asarray(inputs["x"], dtype=np.float32)
    outs, _ = run_on_cores(x, folds)
    return tuple(outs)


# revision 11
# speedup vs baseline: 1.4482x; 1.4482x over previous
"""Trainium2 Bass kernel for nn_Network_47502338294626 (gnn_message_passing).

Strategy: pure data parallel over the batch dim across 8 NeuronCores.
The whole per-row network folds into ~12 fused feature-major matmul stages
(float32r, N=512 batch columns) plus a batch-major "terminal" phase that
emits all 5 outputs through one PSUM bank per 128-row chunk, giving
contiguous row-major DMA writes with no transposes of the big outputs.
All biases ride on "ones" rows carried inside the state tiles, so every
PSUM evacuation is a single activation/copy op.
"""

import numpy as np
from contextlib import ExitStack

B_TOTAL = 131072
N_CORES = 8
ROWS_PER_CORE = B_TOTAL // N_CORES
NB = 512                     # batch columns per feature-major block

# terminal TP column layout (one PSUM bank; all matmul col offsets even)
# [l1in 0:130 | l4in 130:180 | l2o 180 (pad 181) | o6 182:184 |
#  decoded 184:314 | d4 314:364 | l40-42 364:367 | zero-pad ..440]
TP_W = 368
TP_PAD = 440
TC_L1, TC_L4, TC_L2O, TC_O6, TC_DEC, TC_D4, TC_H3 = 0, 130, 180, 182, 184, 314, 364

F32R_WEIGHTS = [
    "WZ1", "WD1A", "WO2AB", "WO2X", "WO3", "WO4O", "WO4X",
    "WY4", "WO5", "WZ6O", "WZ6X",
    "WT_X", "WT_A", "WT_C", "WT_D",
]

WEIGHT_SHAPES = {
    "WZ1": (14, 121), "WD1A": (121, 128),
    "WO2AB": (121, 98), "WO2X": (14, 98), "WO3": (71, 103),
    "WO4O": (103, 51), "WO4X": (14, 51), "WY4": (51, 54),
    "WO5": (51, 101), "WZ6O": (101, 2), "WZ6X": (14, 2),
    "WT_X": (66, 440),
    "WT_A": (128, 256), "WT_C": (103, 52), "WT_D": (54, 54),
}


def round_f32r(a):
    """Round fp32 to the PE's FP32R format (8-bit exp, 11-bit mantissa, RNE)."""
    u = np.asarray(a, np.float32).view(np.uint32).astype(np.uint64)
    lsb = (u >> 12) & 1
    u = (u + 0x7FF + lsb) & ~np.uint64(0xFFF)
    return u.astype(np.uint32).view(np.float32)


# --------------------------------------------------------------------------
# fold-matrix construction (pure numpy; math validated vs the jax reference)
# --------------------------------------------------------------------------
def build_folds(inp):
    f32 = np.float32
    g = {k: np.asarray(v, dtype=np.float64) for k, v in inp.items()}
    w0, b0 = g["w0"], g["b0"]
    E1, E2, E4, E6 = g["E1"], g["E2"], g["E4"], g["E6"]
    W1, b1 = g["W1"], g["b1"]
    W2, b2 = g["W2"], g["b2"]
    W3, b3 = g["W3"], g["b3"]
    W4, b4 = g["W4"], g["b4"]
    W5, b5 = g["W5"], g["b5"]
    W6, b6 = g["W6"], g["b6"]
    decW1, decb1, decW2, decb2 = g["decW1"], g["decb1"], g["decW2"], g["decb2"]
    dec4W1, dec4b1, dec4W2, dec4b2 = g["dec4W1"], g["dec4b1"], g["dec4W2"], g["dec4b2"]
    od2W, od2b = g["od2W"], g["od2b"]
    od40W, od40b = g["od40W"], g["od40b"]
    od41W, od41b = g["od41W"], g["od41b"]
    od42W, od42b = g["od42W"], g["od42b"]

    lrelu = lambda v: np.where(v >= 0, v, 0.01 * v)

    V1 = W1.sum(-1)
    R2 = W2.sum(-1)
    R4 = W4.sum(-1)
    R6 = W6.sum(-1)[:, 0]
    W2m = W2.transpose(2, 0, 1).reshape(10, 70)
    W3m = W3.transpose(2, 0, 1).reshape(10, 100)
    W4m = W4.transpose(2, 0, 1).reshape(10, 50)
    W5m = W5.transpose(2, 0, 1).reshape(10, 100)
    W6m = W6[:, 0, :].T

    var1 = [n for n in range(13) if np.abs(E1[:, n]).sum() > 0]
    const1 = [n for n in range(13) if n not in var1]
    nv = len(var1)
    assert nv == 12, f"unexpected E1 sparsity: {var1}"

    A1x_full = np.zeros((13, 130))
    c1_full = np.zeros(130)
    b0E1 = b0 @ E1
    for n in range(13):
        for o in range(10):
            A1x_full[:, 10 * n + o] = w0 * E1[:, n] * V1[n, o]
            c1_full[10 * n + o] = b0E1[n] * V1[n, o] + b1[n, o]
    var_rows = [10 * n + o for n in var1 for o in range(10)]
    const_rows = [10 * n + o for n in const1 for o in range(10)]
    A1x = A1x_full[:, var_rows]
    c1 = c1_full[var_rows]
    out1_const = lrelu(c1_full[const_rows])

    D1 = decW1.T
    D1v = D1[var_rows, :]
    bias_y1 = decb1 + out1_const @ D1[const_rows, :]

    S1v = np.zeros((120, 10))
    for vi in range(nv):
        for o in range(10):
            S1v[10 * vi + o, o] = 1.0
    sc1 = np.zeros(10)
    for ci in range(len(const1)):
        for o in range(10):
            sc1[o] += out1_const[10 * ci + o]

    b0E2 = b0 @ E2
    A2x = np.zeros((13, 70))
    c2 = np.zeros(70)
    for n in range(7):
        for o in range(10):
            A2x[:, 10 * n + o] = w0 * E2[:, n] * R2[n, o]
            c2[10 * n + o] = b0E2[n] * R2[n, o] + b2[n, o]
    c2 = c2 + sc1 @ W2m
    A2o = S1v @ W2m

    S2 = np.zeros((70, 10))
    for n in range(7):
        for o in range(10):
            S2[10 * n + o, o] = 1.0
    A3 = S2 @ W3m
    c3 = b3.reshape(100).copy()
    A3l2o = np.zeros((70, 1))
    A3l2o[30:40, 0] = od2W[0, :]

    S3 = np.zeros((100, 10))
    for n in range(10):
        for o in range(10):
            S3[10 * n + o, o] = 1.0
    b0E4 = b0 @ E4
    A4x = np.zeros((13, 50))
    c4 = np.zeros(50)
    W4inx = np.zeros((13, 50))
    c_l4 = np.zeros(50)
    for n in range(5):
        for o in range(10):
            A4x[:, 10 * n + o] = w0 * E4[:, n] * R4[n, o]
            c4[10 * n + o] = b0E4[n] * R4[n, o] + b4[n, o]
            W4inx[:, 10 * n + o] = w0 * E4[:, n]
            c_l4[10 * n + o] = b0E4[n]
    A4o = S3 @ W4m
    T4 = np.zeros((10, 50))
    for n in range(5):
        for i in range(10):
            T4[i, 10 * n + i] = 1.0
    W4ino = S3 @ T4

    B4 = dec4W1.T
    d4 = dec4b1
    H4 = np.zeros((50, 3))
    H4[0:10, 0] = od40W[0, :]
    H4[10:20, 1] = od41W[0, :]
    H4[20:30, 2] = od42W[0, :]
    hb = np.array([od40b[0], od41b[0], od42b[0]])
    S4 = np.zeros((50, 10))
    for n in range(5):
        for o in range(10):
            S4[10 * n + o, o] = 1.0
    A5 = S4 @ W5m
    c5 = b5.reshape(100).copy()

    S5 = np.zeros((100, 10))
    for n in range(10):
        for o in range(10):
            S5[10 * n + o, o] = 1.0
    b0E6 = b0 @ E6
    A6x = np.zeros((13, 2))
    c6 = np.zeros(2)
    for n in range(2):
        A6x[:, n] = w0 * E6[:, n] * R6[n]
        c6[n] = b0E6[n] * R6[n] + b6[n, 0]
    A6o = S5 @ W6m

    W1inx = np.zeros((13, 130))
    c_l1 = np.zeros(130)
    for n in range(13):
        for i in range(10):
            W1inx[:, 10 * n + i] = w0 * E1[:, n]
            c_l1[10 * n + i] = b0E1[n]

    D2 = decW2.T
    D4 = dec4W2.T

    F = {}

    def mk(name, arr):
        a = np.ascontiguousarray(np.asarray(arr, dtype=f32))
        assert a.shape == WEIGHT_SHAPES[name], (name, a.shape)
        F[name] = a

    WZ1 = np.zeros((14, 121))
    WZ1[0:13, 0:120] = A1x
    WZ1[13, 0:120] = c1
    WZ1[13, 120] = 1.0
    mk("WZ1", WZ1)

    WD1A = np.zeros((121, 128))
    WD1A[0:120, :] = D1v[:, 0:128]
    WD1A[120, :] = bias_y1[0:128]
    mk("WD1A", WD1A)

    # pC psum [98]: rows 0:70 out2, 70 ones, 71:96 zero, 96:98 y1 tail
    WO2AB = np.zeros((121, 98))
    WO2AB[0:120, 0:70] = A2o
    WO2AB[0:120, 96:98] = D1v[:, 128:130]
    WO2AB[120, 96:98] = bias_y1[128:130]
    mk("WO2AB", WO2AB)

    WO2X = np.zeros((14, 98))
    WO2X[0:13, 0:70] = A2x
    WO2X[13, 0:70] = c2
    WO2X[13, 70] = 1.0
    mk("WO2X", WO2X)

    # OT3 [103]: 0:100 out3 lrelu, 100 lrelu(l2o_pre), 101 lrelu(-l2o_pre),
    # 102 ones.  relu(v) = a*lrelu(v) + b*lrelu(-v), a=1/0.9999, b=0.01/0.9999
    WO3 = np.zeros((71, 103))
    WO3[0:70, 0:100] = A3
    WO3[70, 0:100] = c3
    WO3[0:70, 100:101] = A3l2o
    WO3[70, 100] = od2b[0]
    WO3[0:70, 101:102] = -A3l2o
    WO3[70, 101] = -od2b[0]
    WO3[70, 102] = 1.0
    mk("WO3", WO3)

    WO4O = np.zeros((103, 51))
    WO4O[0:100, 0:50] = A4o
    WO4O[102, 0:50] = c4
    WO4O[102, 50] = 1.0
    mk("WO4O", WO4O)

    WO4X = np.zeros((14, 51))
    WO4X[0:13, 0:50] = A4x
    mk("WO4X", WO4X)

    # Y4H [54]: 0:50 relu y4, 50:53 relu heads40-42, 53 ones
    WY4 = np.zeros((51, 54))
    WY4[0:50, 0:50] = B4
    WY4[0:50, 50:53] = H4
    WY4[50, 0:50] = d4
    WY4[50, 50:53] = hb
    WY4[50, 53] = 1.0
    mk("WY4", WY4)

    WO5 = np.zeros((51, 101))
    WO5[0:50, 0:100] = A5
    WO5[50, 0:100] = c5
    WO5[50, 100] = 1.0
    mk("WO5", WO5)

    WZ6O = np.zeros((101, 2))
    WZ6O[0:100, :] = A6o
    WZ6O[100, :] = c6
    mk("WZ6O", WZ6O)

    WZ6X = np.zeros((14, 2))
    WZ6X[0:13, :] = A6x
    mk("WZ6X", WZ6X)

    # XT66 stationary rows: 0:13 x, 13 ones, 14:32 zero, 32:34 out6,
    # 34:64 zero, 64:66 relu y1-tail
    WT_X = np.zeros((66, 440))
    WT_X[0:13, TC_L1:TC_L1 + 130] = W1inx
    WT_X[13, TC_L1:TC_L1 + 130] = c_l1
    WT_X[0:13, TC_L4:TC_L4 + 50] = W4inx
    WT_X[13, TC_L4:TC_L4 + 50] = c_l4
    WT_X[32, TC_O6] = 1.0
    WT_X[33, TC_O6 + 1] = 1.0
    WT_X[13, TC_DEC:TC_DEC + 130] = decb2
    WT_X[64:66, TC_DEC:TC_DEC + 130] = D2[128:130, :]
    mk("WT_X", WT_X)

    WT_A = np.zeros((128, 256))
    WT_A[:, 0:130] = D2[0:128, :]
    mk("WT_A", WT_A)

    # OT3 stationary: l4in-o (50 cols) + l2o reconstruction (1 col)
    al = 1.0 / 0.9999
    be = 0.01 / 0.9999
    WT_C = np.zeros((103, 52))
    WT_C[0:100, 0:50] = W4ino
    WT_C[100, 50] = al
    WT_C[101, 50] = be
    mk("WT_C", WT_C)

    # Y4H stationary: decoded4 (50) + heads l40-42 (3)
    WT_D = np.zeros((54, 54))
    WT_D[0:50, 0:50] = D4
    WT_D[53, 0:50] = dec4b2
    WT_D[50, 50] = 1.0
    WT_D[51, 51] = 1.0
    WT_D[52, 52] = 1.0
    mk("WT_D", WT_D)

    for name in F32R_WEIGHTS:
        F[name] = round_f32r(F[name])
    return F


# --------------------------------------------------------------------------
# bass kernel
# --------------------------------------------------------------------------
def build_kernel(rows, use_prelu=True):
    import concourse.bacc as bacc
    import concourse.tile as tile
    from concourse import mybir

    f32 = mybir.dt.float32
    f32r = mybir.dt.float32r
    nblk = rows // NB
    assert rows % NB == 0

    nc = bacc.Bacc("TRN2", target_bir_lowering=False, debug=False)

    x_d = nc.dram_tensor("x", [rows, 13], f32, kind="ExternalInput")
    ident_d = nc.dram_tensor("ident", [128, 128], f32, kind="ExternalInput")
    ones_d = nc.dram_tensor("ones", [1, NB], f32r, kind="ExternalInput")
    zeros_d = nc.dram_tensor("zeros", [50, NB], f32r, kind="ExternalInput")
    w_d = {
        name: nc.dram_tensor(
            name, list(shape), f32r if name in F32R_WEIGHTS else f32,
            kind="ExternalInput")
        for name, shape in WEIGHT_SHAPES.items()
    }
    out_d = nc.dram_tensor("out", [rows, 6], f32, kind="ExternalOutput")
    dec_d = nc.dram_tensor("decoded", [rows, 130], f32, kind="ExternalOutput")
    l1_d = nc.dram_tensor("l1in", [rows, 130], f32, kind="ExternalOutput")
    d4_d = nc.dram_tensor("dec4", [rows, 50], f32, kind="ExternalOutput")
    l4_d = nc.dram_tensor("l4in", [rows, 50], f32, kind="ExternalOutput")

    AF = mybir.ActivationFunctionType
    ALU = mybir.AluOpType

    with tile.TileContext(nc) as tc, ExitStack() as ctx:
        wpool = ctx.enter_context(tc.tile_pool(name="weights", bufs=1))
        sb = ctx.enter_context(tc.tile_pool(name="state", bufs=3))
        tout_pool = ctx.enter_context(tc.tile_pool(name="tout", bufs=2))
        xb_pool = ctx.enter_context(tc.tile_pool(name="xb", bufs=3))
        ps = ctx.enter_context(tc.tile_pool(name="ps", bufs=6, space="PSUM"))
        tps = ctx.enter_context(tc.tile_pool(name="tps", bufs=2, space="PSUM"))

        W = {}
        for name, shape in WEIGHT_SHAPES.items():
            t = wpool.tile(list(shape), f32r if name in F32R_WEIGHTS else f32,
                           tag=name)
            nc.sync.dma_start(t[:], w_d[name][:])
            W[name] = t
        ident = wpool.tile([128, 128], f32, tag="ident")
        nc.sync.dma_start(ident[:], ident_d[:])

        def act(out_ap, in_ap):
            if use_prelu:
                nc.scalar.activation(out_ap, in_ap, AF.Lrelu, bias=0.0,
                                     scale=1.0, alpha=0.01)
            else:
                nc.scalar.activation(out_ap, in_ap, AF.Relu, bias=0.0,
                                     scale=1.0)

        for b in range(nblk):
            r0 = b * NB
            # ---- load x block, transpose to feature-major -----------------
            xb = xb_pool.tile([128, 52], f32, tag="xb")
            nc.sync.dma_start(
                xb[:].rearrange("p (c f) -> p c f", c=4),
                x_d[r0:r0 + NB, :].rearrange("(c p) f -> p c f", p=128),
            )
            pX = ps.tile([13, NB], f32, tag="ps")
            for c in range(4):
                nc.tensor.transpose(
                    pX[0:13, c * 128:(c + 1) * 128],
                    xb[:, c * 13:(c + 1) * 13],
                    ident[:],
                )
            XT = sb.tile([66, NB], f32r, tag="XT")
            nc.vector.tensor_copy(XT[0:13, :], pX[0:13, :])
            nc.sync.dma_start(XT[13:14, :], ones_d[:])
            nc.sync.dma_start(XT[14:64, :], zeros_d[:])

            # ---- feature-major chain (float32r matmuls, N=512) ------------
            p1 = ps.tile([121, NB], f32, tag="ps")
            nc.tensor.matmul(p1[:], W["WZ1"][:], XT[0:14, :], start=True, stop=True)
            OUT1 = sb.tile([121, NB], f32r, tag="OUT1")
            act(OUT1[:], p1[:])

            pA = ps.tile([128, NB], f32, tag="ps")
            nc.tensor.matmul(pA[:], W["WD1A"][:], OUT1[:], start=True, stop=True)
            Y1A = sb.tile([128, NB], f32r, tag="Y1A")
            nc.vector.tensor_scalar(Y1A[:], pA[:], 0.0, None, ALU.max)

            pC = ps.tile([98, NB], f32, tag="ps")
            nc.tensor.matmul(pC[:], W["WO2AB"][:], OUT1[:], start=True, stop=False)
            nc.tensor.matmul(pC[:], W["WO2X"][:], XT[0:14, :], start=False, stop=True)
            OUT2 = sb.tile([71, NB], f32r, tag="OUT2")
            act(OUT2[:], pC[0:71, :])
            nc.vector.tensor_scalar(XT[64:66, :], pC[96:98, :], 0.0, None, ALU.max)

            pD = ps.tile([103, NB], f32, tag="ps")
            nc.tensor.matmul(pD[:], W["WO3"][:], OUT2[:], start=True, stop=True)
            OT3 = sb.tile([103, NB], f32r, tag="OT3")
            act(OT3[:], pD[:])

            pE = ps.tile([51, NB], f32, tag="ps")
            nc.tensor.matmul(pE[:], W["WO4O"][:], OT3[:], start=True, stop=False)
            nc.tensor.matmul(pE[:], W["WO4X"][:], XT[0:14, :], start=False, stop=True)
            OUT4 = sb.tile([51, NB], f32r, tag="OUT4")
            act(OUT4[:], pE[:])

            pF = ps.tile([54, NB], f32, tag="ps")
            nc.tensor.matmul(pF[:], W["WY4"][:], OUT4[:], start=True, stop=True)
            Y4H = sb.tile([54, NB], f32r, tag="Y4H")
            nc.vector.tensor_scalar(Y4H[:], pF[:], 0.0, None, ALU.max)

            pG = ps.tile([101, NB], f32, tag="ps")
            nc.tensor.matmul(pG[:], W["WO5"][:], OUT4[:], start=True, stop=True)
            OUT5 = sb.tile([101, NB], f32r, tag="OUT5")
            act(OUT5[:], pG[:])

            pH = ps.tile([2, NB], f32, tag="ps")
            nc.tensor.matmul(pH[:], W["WZ6O"][:], OUT5[:], start=True, stop=False)
            nc.tensor.matmul(pH[:], W["WZ6X"][:], XT[0:14, :], start=False, stop=True)
            act(XT[32:34, :], pH[:])

            # ---- terminal batch-major phase -------------------------------
            TOUT = tout_pool.tile([128, 4 * TP_W], f32, tag="TOUT")
            for c in range(4):
                cs = slice(c * 128, (c + 1) * 128)
                tp = tps.tile([128, TP_PAD], f32, tag="tp")
                nc.tensor.matmul(tp[:, 0:TP_PAD], XT[:, cs], W["WT_X"][:],
                                 start=True, stop=False)
                nc.tensor.matmul(tp[:, TC_DEC:TC_DEC + 256],
                                 Y1A[:, cs], W["WT_A"][:],
                                 start=False, stop=False)
                nc.tensor.matmul(tp[:, TC_L4:TC_L4 + 52],
                                 OT3[:, cs], W["WT_C"][:],
                                 start=False, stop=False)
                nc.tensor.matmul(tp[:, TC_D4:TC_D4 + 54],
                                 Y4H[:, cs], W["WT_D"][:],
                                 start=False, stop=True)
                dst = TOUT[:, c * TP_W:(c + 1) * TP_W]
                if c % 2 == 0:
                    nc.vector.tensor_copy(dst, tp[:, 0:TP_W])
                else:
                    nc.scalar.copy(dst, tp[:, 0:TP_W])

            # ---- output DMAs ---------------------------------------------
            tv = TOUT[:].rearrange("p (c f) -> p c f", c=4)
            for dram, cols, c0, w in (
                (dec_d, slice(0, 130), TC_DEC, 130),
                (l1_d, slice(0, 130), TC_L1, 130),
                (l4_d, slice(0, 50), TC_L4, 50),
                (d4_d, slice(0, 50), TC_D4, 50),
                (out_d, slice(0, 1), TC_L2O, 1),
                (out_d, slice(1, 4), TC_H3, 3),
                (out_d, slice(4, 6), TC_O6, 2),
            ):
                nc.sync.dma_start(
                    dram[r0:r0 + NB, cols].rearrange("(c p) f -> p c f", p=128),
                    tv[:, :, c0:c0 + w],
                )

    nc.compile()
    return nc


_KERNEL_CACHE = {}


def _get_kernel(rows, use_prelu=True):
    key = (rows, use_prelu)
    if key not in _KERNEL_CACHE:
        _KERNEL_CACHE[key] = build_kernel(rows, use_prelu)
    return _KERNEL_CACHE[key]


def run_on_cores(x_full, folds, n_cores=N_CORES, trace=False, use_prelu=True):
    from concourse.bass_utils import run_bass_kernel_spmd

    x_full = np.ascontiguousarray(np.asarray(x_full, dtype=np.float32))
    total = x_full.shape[0]
    rows = total // n_cores
    assert rows * n_cores == total
    nc = _get_kernel(rows, use_prelu)

    base = {name: folds[name] for name in WEIGHT_SHAPES}
    base["ident"] = np.eye(128, dtype=np.float32)
    base["ones"] = np.ones((1, NB), dtype=np.float32)
    base["zeros"] = np.zeros((50, NB), dtype=np.float32)
    in_maps = []
    for i in range(n_cores):
        m = dict(base)
        m["x"] = x_full[i * rows:(i + 1) * rows]
        in_maps.append(m)

    res = run_bass_kernel_spmd(nc, in_maps, list(range(n_cores)), trace=trace)
    outs = []
    for name in ("out", "decoded", "l1in", "dec4", "l4in"):
        outs.append(np.concatenate([res.results[i][name] for i in range(n_cores)],
                                   axis=0))
    return outs, res


def kernel(**inputs):
    folds = build_folds(inputs)
    x = np.asarray(inputs["x"], dtype=np.float32)
    outs, _ = run_on_cores(x, folds)
    return tuple(outs)


# revision 12
# speedup vs baseline: 1.5005x; 1.0362x over previous
"""Trainium2 Bass kernel for nn_Network_47502338294626 (gnn_message_passing).

Strategy: pure data parallel over the batch dim across 8 NeuronCores.
The whole per-row network folds into ~12 fused feature-major matmul stages
(float32r, N=512 batch columns) plus a batch-major "terminal" phase that
emits all 5 outputs through one PSUM bank per 128-row chunk, giving
contiguous row-major DMA writes with no transposes of the big outputs.
All biases ride on "ones" rows carried inside the state tiles, so every
PSUM evacuation is a single activation/copy op.
"""

import numpy as np
from contextlib import ExitStack

B_TOTAL = 131072
N_CORES = 8
ROWS_PER_CORE = B_TOTAL // N_CORES
NB = 512                     # batch columns per feature-major block

# terminal TP column layout (one PSUM bank; all matmul col offsets even)
# [l1in 0:130 | l4in 130:180 | out6 180:186 (l2o,l40,l41,l42,o60,o61) |
#  d4 186:236 | decoded 236:366 | zero-pad 366:492]
TP_W = 366
TP_PAD = 492
TC_L1, TC_L4, TC_OUT, TC_D4, TC_DEC = 0, 130, 180, 186, 236
# TOUT per-output column blocks (4 chunks interleaved row%4)
TO_DEC, TO_L1, TO_L4, TO_D4, TO_OUT = 0, 520, 1040, 1240, 1440

F32R_WEIGHTS = [
    "WZ1", "WD1A", "WO2AB", "WO2X", "WO3", "WO4O", "WO4X",
    "WY4", "WO5", "WZ6O", "WZ6X",
    "WT_X", "WT_A", "WT_C", "WT_D",
]

WEIGHT_SHAPES = {
    "WZ1": (14, 121), "WD1A": (121, 128),
    "WO2AB": (121, 98), "WO2X": (14, 98), "WO3": (71, 103),
    "WO4O": (103, 51), "WO4X": (14, 51), "WY4": (51, 54),
    "WO5": (51, 101), "WZ6O": (101, 2), "WZ6X": (14, 2),
    "WT_X": (66, 492),
    "WT_A": (128, 256), "WT_C": (103, 52), "WT_D": (54, 56),
}


def round_f32r(a):
    """Round fp32 to the PE's FP32R format (8-bit exp, 11-bit mantissa, RNE)."""
    u = np.asarray(a, np.float32).view(np.uint32).astype(np.uint64)
    lsb = (u >> 12) & 1
    u = (u + 0x7FF + lsb) & ~np.uint64(0xFFF)
    return u.astype(np.uint32).view(np.float32)


# --------------------------------------------------------------------------
# fold-matrix construction (pure numpy; math validated vs the jax reference)
# --------------------------------------------------------------------------
def build_folds(inp):
    f32 = np.float32
    g = {k: np.asarray(v, dtype=np.float64) for k, v in inp.items()}
    w0, b0 = g["w0"], g["b0"]
    E1, E2, E4, E6 = g["E1"], g["E2"], g["E4"], g["E6"]
    W1, b1 = g["W1"], g["b1"]
    W2, b2 = g["W2"], g["b2"]
    W3, b3 = g["W3"], g["b3"]
    W4, b4 = g["W4"], g["b4"]
    W5, b5 = g["W5"], g["b5"]
    W6, b6 = g["W6"], g["b6"]
    decW1, decb1, decW2, decb2 = g["decW1"], g["decb1"], g["decW2"], g["decb2"]
    dec4W1, dec4b1, dec4W2, dec4b2 = g["dec4W1"], g["dec4b1"], g["dec4W2"], g["dec4b2"]
    od2W, od2b = g["od2W"], g["od2b"]
    od40W, od40b = g["od40W"], g["od40b"]
    od41W, od41b = g["od41W"], g["od41b"]
    od42W, od42b = g["od42W"], g["od42b"]

    lrelu = lambda v: np.where(v >= 0, v, 0.01 * v)

    V1 = W1.sum(-1)
    R2 = W2.sum(-1)
    R4 = W4.sum(-1)
    R6 = W6.sum(-1)[:, 0]
    W2m = W2.transpose(2, 0, 1).reshape(10, 70)
    W3m = W3.transpose(2, 0, 1).reshape(10, 100)
    W4m = W4.transpose(2, 0, 1).reshape(10, 50)
    W5m = W5.transpose(2, 0, 1).reshape(10, 100)
    W6m = W6[:, 0, :].T

    var1 = [n for n in range(13) if np.abs(E1[:, n]).sum() > 0]
    const1 = [n for n in range(13) if n not in var1]
    nv = len(var1)
    assert nv == 12, f"unexpected E1 sparsity: {var1}"

    A1x_full = np.zeros((13, 130))
    c1_full = np.zeros(130)
    b0E1 = b0 @ E1
    for n in range(13):
        for o in range(10):
            A1x_full[:, 10 * n + o] = w0 * E1[:, n] * V1[n, o]
            c1_full[10 * n + o] = b0E1[n] * V1[n, o] + b1[n, o]
    var_rows = [10 * n + o for n in var1 for o in range(10)]
    const_rows = [10 * n + o for n in const1 for o in range(10)]
    A1x = A1x_full[:, var_rows]
    c1 = c1_full[var_rows]
    out1_const = lrelu(c1_full[const_rows])

    D1 = decW1.T
    D1v = D1[var_rows, :]
    bias_y1 = decb1 + out1_const @ D1[const_rows, :]

    S1v = np.zeros((120, 10))
    for vi in range(nv):
        for o in range(10):
            S1v[10 * vi + o, o] = 1.0
    sc1 = np.zeros(10)
    for ci in range(len(const1)):
        for o in range(10):
            sc1[o] += out1_const[10 * ci + o]

    b0E2 = b0 @ E2
    A2x = np.zeros((13, 70))
    c2 = np.zeros(70)
    for n in range(7):
        for o in range(10):
            A2x[:, 10 * n + o] = w0 * E2[:, n] * R2[n, o]
            c2[10 * n + o] = b0E2[n] * R2[n, o] + b2[n, o]
    c2 = c2 + sc1 @ W2m
    A2o = S1v @ W2m

    S2 = np.zeros((70, 10))
    for n in range(7):
        for o in range(10):
            S2[10 * n + o, o] = 1.0
    A3 = S2 @ W3m
    c3 = b3.reshape(100).copy()
    A3l2o = np.zeros((70, 1))
    A3l2o[30:40, 0] = od2W[0, :]

    S3 = np.zeros((100, 10))
    for n in range(10):
        for o in range(10):
            S3[10 * n + o, o] = 1.0
    b0E4 = b0 @ E4
    A4x = np.zeros((13, 50))
    c4 = np.zeros(50)
    W4inx = np.zeros((13, 50))
    c_l4 = np.zeros(50)
    for n in range(5):
        for o in range(10):
            A4x[:, 10 * n + o] = w0 * E4[:, n] * R4[n, o]
            c4[10 * n + o] = b0E4[n] * R4[n, o] + b4[n, o]
            W4inx[:, 10 * n + o] = w0 * E4[:, n]
            c_l4[10 * n + o] = b0E4[n]
    A4o = S3 @ W4m
    T4 = np.zeros((10, 50))
    for n in range(5):
        for i in range(10):
            T4[i, 10 * n + i] = 1.0
    W4ino = S3 @ T4

    B4 = dec4W1.T
    d4 = dec4b1
    H4 = np.zeros((50, 3))
    H4[0:10, 0] = od40W[0, :]
    H4[10:20, 1] = od41W[0, :]
    H4[20:30, 2] = od42W[0, :]
    hb = np.array([od40b[0], od41b[0], od42b[0]])
    S4 = np.zeros((50, 10))
    for n in range(5):
        for o in range(10):
            S4[10 * n + o, o] = 1.0
    A5 = S4 @ W5m
    c5 = b5.reshape(100).copy()

    S5 = np.zeros((100, 10))
    for n in range(10):
        for o in range(10):
            S5[10 * n + o, o] = 1.0
    b0E6 = b0 @ E6
    A6x = np.zeros((13, 2))
    c6 = np.zeros(2)
    for n in range(2):
        A6x[:, n] = w0 * E6[:, n] * R6[n]
        c6[n] = b0E6[n] * R6[n] + b6[n, 0]
    A6o = S5 @ W6m

    W1inx = np.zeros((13, 130))
    c_l1 = np.zeros(130)
    for n in range(13):
        for i in range(10):
            W1inx[:, 10 * n + i] = w0 * E1[:, n]
            c_l1[10 * n + i] = b0E1[n]

    D2 = decW2.T
    D4 = dec4W2.T

    F = {}

    def mk(name, arr):
        a = np.ascontiguousarray(np.asarray(arr, dtype=f32))
        assert a.shape == WEIGHT_SHAPES[name], (name, a.shape)
        F[name] = a

    WZ1 = np.zeros((14, 121))
    WZ1[0:13, 0:120] = A1x
    WZ1[13, 0:120] = c1
    WZ1[13, 120] = 1.0
    mk("WZ1", WZ1)

    WD1A = np.zeros((121, 128))
    WD1A[0:120, :] = D1v[:, 0:128]
    WD1A[120, :] = bias_y1[0:128]
    mk("WD1A", WD1A)

    # pC psum [98]: rows 0:70 out2, 70 ones, 71:96 zero, 96:98 y1 tail
    WO2AB = np.zeros((121, 98))
    WO2AB[0:120, 0:70] = A2o
    WO2AB[0:120, 96:98] = D1v[:, 128:130]
    WO2AB[120, 96:98] = bias_y1[128:130]
    mk("WO2AB", WO2AB)

    WO2X = np.zeros((14, 98))
    WO2X[0:13, 0:70] = A2x
    WO2X[13, 0:70] = c2
    WO2X[13, 70] = 1.0
    mk("WO2X", WO2X)

    # OT3 [103]: 0:100 out3 lrelu, 100 lrelu(l2o_pre), 101 lrelu(-l2o_pre),
    # 102 ones.  relu(v) = a*lrelu(v) + b*lrelu(-v), a=1/0.9999, b=0.01/0.9999
    WO3 = np.zeros((71, 103))
    WO3[0:70, 0:100] = A3
    WO3[70, 0:100] = c3
    WO3[0:70, 100:101] = A3l2o
    WO3[70, 100] = od2b[0]
    WO3[0:70, 101:102] = -A3l2o
    WO3[70, 101] = -od2b[0]
    WO3[70, 102] = 1.0
    mk("WO3", WO3)

    WO4O = np.zeros((103, 51))
    WO4O[0:100, 0:50] = A4o
    WO4O[102, 0:50] = c4
    WO4O[102, 50] = 1.0
    mk("WO4O", WO4O)

    WO4X = np.zeros((14, 51))
    WO4X[0:13, 0:50] = A4x
    mk("WO4X", WO4X)

    # Y4H [54]: 0:50 relu y4, 50:53 relu heads40-42, 53 ones
    WY4 = np.zeros((51, 54))
    WY4[0:50, 0:50] = B4
    WY4[0:50, 50:53] = H4
    WY4[50, 0:50] = d4
    WY4[50, 50:53] = hb
    WY4[50, 53] = 1.0
    mk("WY4", WY4)

    WO5 = np.zeros((51, 101))
    WO5[0:50, 0:100] = A5
    WO5[50, 0:100] = c5
    WO5[50, 100] = 1.0
    mk("WO5", WO5)

    WZ6O = np.zeros((101, 2))
    WZ6O[0:100, :] = A6o
    WZ6O[100, :] = c6
    mk("WZ6O", WZ6O)

    WZ6X = np.zeros((14, 2))
    WZ6X[0:13, :] = A6x
    mk("WZ6X", WZ6X)

    # XT66 stationary rows: 0:13 x, 13 ones, 14:32 zero, 32:34 out6,
    # 34:64 zero, 64:66 relu y1-tail
    WT_X = np.zeros((66, 492))
    WT_X[0:13, TC_L1:TC_L1 + 130] = W1inx
    WT_X[13, TC_L1:TC_L1 + 130] = c_l1
    WT_X[0:13, TC_L4:TC_L4 + 50] = W4inx
    WT_X[13, TC_L4:TC_L4 + 50] = c_l4
    WT_X[32, TC_OUT + 4] = 1.0
    WT_X[33, TC_OUT + 5] = 1.0
    WT_X[13, TC_DEC:TC_DEC + 130] = decb2
    WT_X[64:66, TC_DEC:TC_DEC + 130] = D2[128:130, :]
    mk("WT_X", WT_X)

    WT_A = np.zeros((128, 256))
    WT_A[:, 0:130] = D2[0:128, :]
    mk("WT_A", WT_A)

    # OT3 stationary: l4in-o (50 cols) + l2o reconstruction (1 col)
    al = 1.0 / 0.9999
    be = 0.01 / 0.9999
    WT_C = np.zeros((103, 52))
    WT_C[0:100, 0:50] = W4ino
    WT_C[100, 50] = al
    WT_C[101, 50] = be
    mk("WT_C", WT_C)

    # Y4H stationary: decoded4 (50) + heads l40-42 (3)
    # out cols 180:236: [l2o(0) | h3 1:4 | o6 4:6 | d4 6:56]
    WT_D = np.zeros((54, 56))
    WT_D[50, 1] = 1.0
    WT_D[51, 2] = 1.0
    WT_D[52, 3] = 1.0
    WT_D[0:50, 6:56] = D4
    WT_D[53, 6:56] = dec4b2
    mk("WT_D", WT_D)

    for name in F32R_WEIGHTS:
        F[name] = round_f32r(F[name])
    return F


# --------------------------------------------------------------------------
# bass kernel
# --------------------------------------------------------------------------
def build_kernel(rows, use_prelu=True):
    import concourse.bacc as bacc
    import concourse.tile as tile
    from concourse import mybir

    f32 = mybir.dt.float32
    f32r = mybir.dt.float32r
    nblk = rows // NB
    assert rows % NB == 0

    nc = bacc.Bacc("TRN2", target_bir_lowering=False, debug=False)

    x_d = nc.dram_tensor("x", [rows, 13], f32, kind="ExternalInput")
    ident_d = nc.dram_tensor("ident", [128, 128], f32, kind="ExternalInput")
    ones_d = nc.dram_tensor("ones", [1, NB], f32r, kind="ExternalInput")
    zeros_d = nc.dram_tensor("zeros", [50, NB], f32r, kind="ExternalInput")
    w_d = {
        name: nc.dram_tensor(
            name, list(shape), f32r if name in F32R_WEIGHTS else f32,
            kind="ExternalInput")
        for name, shape in WEIGHT_SHAPES.items()
    }
    out_d = nc.dram_tensor("out", [rows, 6], f32, kind="ExternalOutput")
    dec_d = nc.dram_tensor("decoded", [rows, 130], f32, kind="ExternalOutput")
    l1_d = nc.dram_tensor("l1in", [rows, 130], f32, kind="ExternalOutput")
    d4_d = nc.dram_tensor("dec4", [rows, 50], f32, kind="ExternalOutput")
    l4_d = nc.dram_tensor("l4in", [rows, 50], f32, kind="ExternalOutput")

    AF = mybir.ActivationFunctionType
    ALU = mybir.AluOpType

    with tile.TileContext(nc) as tc, ExitStack() as ctx:
        wpool = ctx.enter_context(tc.tile_pool(name="weights", bufs=1))
        sb = ctx.enter_context(tc.tile_pool(name="state", bufs=3))
        tout_pool = ctx.enter_context(tc.tile_pool(name="tout", bufs=2))
        xb_pool = ctx.enter_context(tc.tile_pool(name="xb", bufs=3))
        ps = ctx.enter_context(tc.tile_pool(name="ps", bufs=6, space="PSUM"))
        tps = ctx.enter_context(tc.tile_pool(name="tps", bufs=2, space="PSUM"))

        W = {}
        for name, shape in WEIGHT_SHAPES.items():
            t = wpool.tile(list(shape), f32r if name in F32R_WEIGHTS else f32,
                           tag=name)
            nc.sync.dma_start(t[:], w_d[name][:])
            W[name] = t
        ident = wpool.tile([128, 128], f32, tag="ident")
        nc.sync.dma_start(ident[:], ident_d[:])

        def act(out_ap, in_ap):
            if use_prelu:
                nc.scalar.activation(out_ap, in_ap, AF.Lrelu, bias=0.0,
                                     scale=1.0, alpha=0.01)
            else:
                nc.scalar.activation(out_ap, in_ap, AF.Relu, bias=0.0,
                                     scale=1.0)

        for b in range(nblk):
            r0 = b * NB
            # ---- load x block, transpose to feature-major -----------------
            xb = xb_pool.tile([128, 52], f32, tag="xb")
            nc.sync.dma_start(
                xb[:].rearrange("p (c f) -> p c f", c=4),
                x_d[r0:r0 + NB, :].rearrange("(p c) f -> p c f", c=4),
            )
            pX = ps.tile([13, NB], f32, tag="ps")
            for c in range(4):
                nc.tensor.transpose(
                    pX[0:13, c * 128:(c + 1) * 128],
                    xb[:, c * 13:(c + 1) * 13],
                    ident[:],
                )
            XT = sb.tile([66, NB], f32r, tag="XT")
            nc.vector.tensor_copy(XT[0:13, :], pX[0:13, :])
            nc.sync.dma_start(XT[13:14, :], ones_d[:])
            nc.sync.dma_start(XT[14:64, :], zeros_d[:])

            # ---- feature-major chain (float32r matmuls, N=512) ------------
            p1 = ps.tile([121, NB], f32, tag="ps")
            nc.tensor.matmul(p1[:], W["WZ1"][:], XT[0:14, :], start=True, stop=True)
            OUT1 = sb.tile([121, NB], f32r, tag="OUT1")
            act(OUT1[:], p1[:])

            pA = ps.tile([128, NB], f32, tag="ps")
            nc.tensor.matmul(pA[:], W["WD1A"][:], OUT1[:], start=True, stop=True)
            Y1A = sb.tile([128, NB], f32r, tag="Y1A")
            nc.vector.tensor_scalar(Y1A[:], pA[:], 0.0, None, ALU.max)

            pC = ps.tile([98, NB], f32, tag="ps")
            nc.tensor.matmul(pC[:], W["WO2AB"][:], OUT1[:], start=True, stop=False)
            nc.tensor.matmul(pC[:], W["WO2X"][:], XT[0:14, :], start=False, stop=True)
            OUT2 = sb.tile([71, NB], f32r, tag="OUT2")
            act(OUT2[:], pC[0:71, :])
            nc.vector.tensor_scalar(XT[64:66, :], pC[96:98, :], 0.0, None, ALU.max)

            pD = ps.tile([103, NB], f32, tag="ps")
            nc.tensor.matmul(pD[:], W["WO3"][:], OUT2[:], start=True, stop=True)
            OT3 = sb.tile([103, NB], f32r, tag="OT3")
            act(OT3[:], pD[:])

            pE = ps.tile([51, NB], f32, tag="ps")
            nc.tensor.matmul(pE[:], W["WO4O"][:], OT3[:], start=True, stop=False)
            nc.tensor.matmul(pE[:], W["WO4X"][:], XT[0:14, :], start=False, stop=True)
            OUT4 = sb.tile([51, NB], f32r, tag="OUT4")
            act(OUT4[:], pE[:])

            pF = ps.tile([54, NB], f32, tag="ps")
            nc.tensor.matmul(pF[:], W["WY4"][:], OUT4[:], start=True, stop=True)
            Y4H = sb.tile([54, NB], f32r, tag="Y4H")
            nc.vector.tensor_scalar(Y4H[:], pF[:], 0.0, None, ALU.max)

            pG = ps.tile([101, NB], f32, tag="ps")
            nc.tensor.matmul(pG[:], W["WO5"][:], OUT4[:], start=True, stop=True)
            OUT5 = sb.tile([101, NB], f32r, tag="OUT5")
            act(OUT5[:], pG[:])

            pH = ps.tile([2, NB], f32, tag="ps")
            nc.tensor.matmul(pH[:], W["WZ6O"][:], OUT5[:], start=True, stop=False)
            nc.tensor.matmul(pH[:], W["WZ6X"][:], XT[0:14, :], start=False, stop=True)
            act(XT[32:34, :], pH[:])

            # ---- terminal batch-major phase -------------------------------
            TOUT = tout_pool.tile([128, 4 * TP_W], f32, tag="TOUT")
            for c in range(4):
                cs = slice(c * 128, (c + 1) * 128)
                tp = tps.tile([128, TP_PAD], f32, tag="tp")
                nc.tensor.matmul(tp[:, 0:TP_PAD], XT[:, cs], W["WT_X"][:],
                                 start=True, stop=False)
                nc.tensor.matmul(tp[:, TC_DEC:TC_DEC + 256],
                                 Y1A[:, cs], W["WT_A"][:],
                                 start=False, stop=False)
                nc.tensor.matmul(tp[:, TC_L4:TC_L4 + 52],
                                 OT3[:, cs], W["WT_C"][:],
                                 start=False, stop=False)
                nc.tensor.matmul(tp[:, TC_OUT:TC_OUT + 56],
                                 Y4H[:, cs], W["WT_D"][:],
                                 start=False, stop=True)
                for i, (to, c0, w) in enumerate((
                    (TO_DEC, TC_DEC, 130),
                    (TO_L1, TC_L1, 130),
                    (TO_L4, TC_L4, 50),
                    (TO_D4, TC_D4, 50),
                    (TO_OUT, TC_OUT, 6),
                )):
                    dst = TOUT[:, to + c * w:to + (c + 1) * w]
                    if (c + i) % 2 == 0:
                        nc.vector.tensor_copy(dst, tp[:, c0:c0 + w])
                    else:
                        nc.scalar.copy(dst, tp[:, c0:c0 + w])

            # ---- output DMAs (4 rows per descriptor) ---------------------
            for dram, to, w in (
                (dec_d, TO_DEC, 130),
                (l1_d, TO_L1, 130),
                (l4_d, TO_L4, 50),
                (d4_d, TO_D4, 50),
                (out_d, TO_OUT, 6),
            ):
                nc.sync.dma_start(
                    dram[r0:r0 + NB, :].rearrange("(p c) f -> p (c f)", c=4),
                    TOUT[:, to:to + 4 * w],
                )

    nc.compile()
    return nc


_KERNEL_CACHE = {}


def _get_kernel(rows, use_prelu=True):
    key = (rows, use_prelu)
    if key not in _KERNEL_CACHE:
        _KERNEL_CACHE[key] = build_kernel(rows, use_prelu)
    return _KERNEL_CACHE[key]


def run_on_cores(x_full, folds, n_cores=N_CORES, trace=False, use_prelu=True):
    from concourse.bass_utils import run_bass_kernel_spmd

    x_full = np.ascontiguousarray(np.asarray(x_full, dtype=np.float32))
    total = x_full.shape[0]
    rows = total // n_cores
    assert rows * n_cores == total
    nc = _get_kernel(rows, use_prelu)

    base = {name: folds[name] for name in WEIGHT_SHAPES}
    base["ident"] = np.eye(128, dtype=np.float32)
    base["ones"] = np.ones((1, NB), dtype=np.float32)
    base["zeros"] = np.zeros((50, NB), dtype=np.float32)
    in_maps = []
    for i in range(n_cores):
        m = dict(base)
        m["x"] = x_full[i * rows:(i + 1) * rows]
        in_maps.append(m)

    res = run_bass_kernel_spmd(nc, in_maps, list(range(n_cores)), trace=trace)
    outs = []
    for name in ("out", "decoded", "l1in", "dec4", "l4in"):
        outs.append(np.concatenate([res.results[i][name] for i in range(n_cores)],
                                   axis=0))
    return outs, res


def kernel(**inputs):
    folds = build_folds(inputs)
    x = np.asarray(inputs["x"], dtype=np.float32)
    outs, _ = run_on_cores(x, folds)
    return tuple(outs)


# revision 13
# speedup vs baseline: 2.0117x; 1.3407x over previous
"""Trainium2 Bass kernel for nn_Network_47502338294626 (gnn_message_passing).

Strategy: pure data parallel over the batch dim across 8 NeuronCores.
The whole per-row network folds into ~12 fused feature-major matmul stages
(float32r, N=512 batch columns) plus a batch-major "terminal" phase that
emits all 5 outputs through one PSUM bank per 128-row chunk, giving
contiguous row-major DMA writes with no transposes of the big outputs.
All biases ride on "ones" rows carried inside the state tiles, so every
PSUM evacuation is a single activation/copy op.
"""

import numpy as np
from contextlib import ExitStack

B_TOTAL = 131072
N_CORES = 8
ROWS_PER_CORE = B_TOTAL // N_CORES
NB = 512                     # batch columns per feature-major block

# terminal TP column layout (one PSUM bank; all matmul col offsets even)
# [l1in 0:130 | l4in 130:180 | out6 180:186 (l2o,l40,l41,l42,o60,o61) |
#  d4 186:236 | decoded 236:366 | zero-pad 366:492]
TP_W = 366
TP_PAD = 492
TC_L1, TC_L4, TC_OUT, TC_D4, TC_DEC = 0, 130, 180, 186, 236
# TOUT per-output column blocks (4 chunks interleaved row%4)
TO_DEC, TO_L1, TO_L4, TO_D4, TO_OUT = 0, 520, 1040, 1240, 1440

F32R_WEIGHTS = [
    "WZ1", "WD1A", "WO2AB", "WO2X", "WO3", "WO4O", "WO4X",
    "WY4", "WO5", "WZ6O", "WZ6X",
    "WT_X", "WT_A", "WT_C", "WT_D",
]

WEIGHT_SHAPES = {
    "WZ1": (14, 121), "WD1A": (121, 128),
    "WO2AB": (121, 98), "WO2X": (14, 98), "WO3": (71, 103),
    "WO4O": (103, 51), "WO4X": (14, 51), "WY4": (51, 54),
    "WO5": (51, 101), "WZ6O": (101, 2), "WZ6X": (14, 2),
    "WT_X": (66, 492),
    "WT_A": (128, 256), "WT_C": (103, 52), "WT_D": (54, 56),
}


def round_f32r(a):
    """Round fp32 to the PE's FP32R format (8-bit exp, 11-bit mantissa, RNE)."""
    u = np.asarray(a, np.float32).view(np.uint32).astype(np.uint64)
    lsb = (u >> 12) & 1
    u = (u + 0x7FF + lsb) & ~np.uint64(0xFFF)
    return u.astype(np.uint32).view(np.float32)


# --------------------------------------------------------------------------
# fold-matrix construction (pure numpy; math validated vs the jax reference)
# --------------------------------------------------------------------------
def build_folds(inp):
    f32 = np.float32
    g = {k: np.asarray(v, dtype=np.float64) for k, v in inp.items()}
    w0, b0 = g["w0"], g["b0"]
    E1, E2, E4, E6 = g["E1"], g["E2"], g["E4"], g["E6"]
    W1, b1 = g["W1"], g["b1"]
    W2, b2 = g["W2"], g["b2"]
    W3, b3 = g["W3"], g["b3"]
    W4, b4 = g["W4"], g["b4"]
    W5, b5 = g["W5"], g["b5"]
    W6, b6 = g["W6"], g["b6"]
    decW1, decb1, decW2, decb2 = g["decW1"], g["decb1"], g["decW2"], g["decb2"]
    dec4W1, dec4b1, dec4W2, dec4b2 = g["dec4W1"], g["dec4b1"], g["dec4W2"], g["dec4b2"]
    od2W, od2b = g["od2W"], g["od2b"]
    od40W, od40b = g["od40W"], g["od40b"]
    od41W, od41b = g["od41W"], g["od41b"]
    od42W, od42b = g["od42W"], g["od42b"]

    lrelu = lambda v: np.where(v >= 0, v, 0.01 * v)

    V1 = W1.sum(-1)
    R2 = W2.sum(-1)
    R4 = W4.sum(-1)
    R6 = W6.sum(-1)[:, 0]
    W2m = W2.transpose(2, 0, 1).reshape(10, 70)
    W3m = W3.transpose(2, 0, 1).reshape(10, 100)
    W4m = W4.transpose(2, 0, 1).reshape(10, 50)
    W5m = W5.transpose(2, 0, 1).reshape(10, 100)
    W6m = W6[:, 0, :].T

    var1 = [n for n in range(13) if np.abs(E1[:, n]).sum() > 0]
    const1 = [n for n in range(13) if n not in var1]
    nv = len(var1)
    assert nv == 12, f"unexpected E1 sparsity: {var1}"

    A1x_full = np.zeros((13, 130))
    c1_full = np.zeros(130)
    b0E1 = b0 @ E1
    for n in range(13):
        for o in range(10):
            A1x_full[:, 10 * n + o] = w0 * E1[:, n] * V1[n, o]
            c1_full[10 * n + o] = b0E1[n] * V1[n, o] + b1[n, o]
    var_rows = [10 * n + o for n in var1 for o in range(10)]
    const_rows = [10 * n + o for n in const1 for o in range(10)]
    A1x = A1x_full[:, var_rows]
    c1 = c1_full[var_rows]
    out1_const = lrelu(c1_full[const_rows])

    D1 = decW1.T
    D1v = D1[var_rows, :]
    bias_y1 = decb1 + out1_const @ D1[const_rows, :]

    S1v = np.zeros((120, 10))
    for vi in range(nv):
        for o in range(10):
            S1v[10 * vi + o, o] = 1.0
    sc1 = np.zeros(10)
    for ci in range(len(const1)):
        for o in range(10):
            sc1[o] += out1_const[10 * ci + o]

    b0E2 = b0 @ E2
    A2x = np.zeros((13, 70))
    c2 = np.zeros(70)
    for n in range(7):
        for o in range(10):
            A2x[:, 10 * n + o] = w0 * E2[:, n] * R2[n, o]
            c2[10 * n + o] = b0E2[n] * R2[n, o] + b2[n, o]
    c2 = c2 + sc1 @ W2m
    A2o = S1v @ W2m

    S2 = np.zeros((70, 10))
    for n in range(7):
        for o in range(10):
            S2[10 * n + o, o] = 1.0
    A3 = S2 @ W3m
    c3 = b3.reshape(100).copy()
    A3l2o = np.zeros((70, 1))
    A3l2o[30:40, 0] = od2W[0, :]

    S3 = np.zeros((100, 10))
    for n in range(10):
        for o in range(10):
            S3[10 * n + o, o] = 1.0
    b0E4 = b0 @ E4
    A4x = np.zeros((13, 50))
    c4 = np.zeros(50)
    W4inx = np.zeros((13, 50))
    c_l4 = np.zeros(50)
    for n in range(5):
        for o in range(10):
            A4x[:, 10 * n + o] = w0 * E4[:, n] * R4[n, o]
            c4[10 * n + o] = b0E4[n] * R4[n, o] + b4[n, o]
            W4inx[:, 10 * n + o] = w0 * E4[:, n]
            c_l4[10 * n + o] = b0E4[n]
    A4o = S3 @ W4m
    T4 = np.zeros((10, 50))
    for n in range(5):
        for i in range(10):
            T4[i, 10 * n + i] = 1.0
    W4ino = S3 @ T4

    B4 = dec4W1.T
    d4 = dec4b1
    H4 = np.zeros((50, 3))
    H4[0:10, 0] = od40W[0, :]
    H4[10:20, 1] = od41W[0, :]
    H4[20:30, 2] = od42W[0, :]
    hb = np.array([od40b[0], od41b[0], od42b[0]])
    S4 = np.zeros((50, 10))
    for n in range(5):
        for o in range(10):
            S4[10 * n + o, o] = 1.0
    A5 = S4 @ W5m
    c5 = b5.reshape(100).copy()

    S5 = np.zeros((100, 10))
    for n in range(10):
        for o in range(10):
            S5[10 * n + o, o] = 1.0
    b0E6 = b0 @ E6
    A6x = np.zeros((13, 2))
    c6 = np.zeros(2)
    for n in range(2):
        A6x[:, n] = w0 * E6[:, n] * R6[n]
        c6[n] = b0E6[n] * R6[n] + b6[n, 0]
    A6o = S5 @ W6m

    W1inx = np.zeros((13, 130))
    c_l1 = np.zeros(130)
    for n in range(13):
        for i in range(10):
            W1inx[:, 10 * n + i] = w0 * E1[:, n]
            c_l1[10 * n + i] = b0E1[n]

    D2 = decW2.T
    D4 = dec4W2.T

    F = {}

    def mk(name, arr):
        a = np.ascontiguousarray(np.asarray(arr, dtype=f32))
        assert a.shape == WEIGHT_SHAPES[name], (name, a.shape)
        F[name] = a

    WZ1 = np.zeros((14, 121))
    WZ1[0:13, 0:120] = A1x
    WZ1[13, 0:120] = c1
    WZ1[13, 120] = 1.0
    mk("WZ1", WZ1)

    WD1A = np.zeros((121, 128))
    WD1A[0:120, :] = D1v[:, 0:128]
    WD1A[120, :] = bias_y1[0:128]
    mk("WD1A", WD1A)

    # pC psum [98]: rows 0:70 out2, 70 ones, 71:96 zero, 96:98 y1 tail
    WO2AB = np.zeros((121, 98))
    WO2AB[0:120, 0:70] = A2o
    WO2AB[0:120, 96:98] = D1v[:, 128:130]
    WO2AB[120, 96:98] = bias_y1[128:130]
    mk("WO2AB", WO2AB)

    WO2X = np.zeros((14, 98))
    WO2X[0:13, 0:70] = A2x
    WO2X[13, 0:70] = c2
    WO2X[13, 70] = 1.0
    mk("WO2X", WO2X)

    # OT3 [103]: 0:100 out3 lrelu, 100 lrelu(l2o_pre), 101 lrelu(-l2o_pre),
    # 102 ones.  relu(v) = a*lrelu(v) + b*lrelu(-v), a=1/0.9999, b=0.01/0.9999
    WO3 = np.zeros((71, 103))
    WO3[0:70, 0:100] = A3
    WO3[70, 0:100] = c3
    WO3[0:70, 100:101] = A3l2o
    WO3[70, 100] = od2b[0]
    WO3[0:70, 101:102] = -A3l2o
    WO3[70, 101] = -od2b[0]
    WO3[70, 102] = 1.0
    mk("WO3", WO3)

    WO4O = np.zeros((103, 51))
    WO4O[0:100, 0:50] = A4o
    WO4O[102, 0:50] = c4
    WO4O[102, 50] = 1.0
    mk("WO4O", WO4O)

    WO4X = np.zeros((14, 51))
    WO4X[0:13, 0:50] = A4x
    mk("WO4X", WO4X)

    # Y4H [54]: 0:50 relu y4, 50:53 relu heads40-42, 53 ones
    WY4 = np.zeros((51, 54))
    WY4[0:50, 0:50] = B4
    WY4[0:50, 50:53] = H4
    WY4[50, 0:50] = d4
    WY4[50, 50:53] = hb
    WY4[50, 53] = 1.0
    mk("WY4", WY4)

    WO5 = np.zeros((51, 101))
    WO5[0:50, 0:100] = A5
    WO5[50, 0:100] = c5
    WO5[50, 100] = 1.0
    mk("WO5", WO5)

    WZ6O = np.zeros((101, 2))
    WZ6O[0:100, :] = A6o
    WZ6O[100, :] = c6
    mk("WZ6O", WZ6O)

    WZ6X = np.zeros((14, 2))
    WZ6X[0:13, :] = A6x
    mk("WZ6X", WZ6X)

    # XT66 stationary rows: 0:13 x, 13 ones, 14:32 zero, 32:34 out6,
    # 34:64 zero, 64:66 relu y1-tail
    WT_X = np.zeros((66, 492))
    WT_X[0:13, TC_L1:TC_L1 + 130] = W1inx
    WT_X[13, TC_L1:TC_L1 + 130] = c_l1
    WT_X[0:13, TC_L4:TC_L4 + 50] = W4inx
    WT_X[13, TC_L4:TC_L4 + 50] = c_l4
    WT_X[32, TC_OUT + 4] = 1.0
    WT_X[33, TC_OUT + 5] = 1.0
    WT_X[13, TC_DEC:TC_DEC + 130] = decb2
    WT_X[64:66, TC_DEC:TC_DEC + 130] = D2[128:130, :]
    mk("WT_X", WT_X)

    WT_A = np.zeros((128, 256))
    WT_A[:, 0:130] = D2[0:128, :]
    mk("WT_A", WT_A)

    # OT3 stationary: l4in-o (50 cols) + l2o reconstruction (1 col)
    al = 1.0 / 0.9999
    be = 0.01 / 0.9999
    WT_C = np.zeros((103, 52))
    WT_C[0:100, 0:50] = W4ino
    WT_C[100, 50] = al
    WT_C[101, 50] = be
    mk("WT_C", WT_C)

    # Y4H stationary: decoded4 (50) + heads l40-42 (3)
    # out cols 180:236: [l2o(0) | h3 1:4 | o6 4:6 | d4 6:56]
    WT_D = np.zeros((54, 56))
    WT_D[50, 1] = 1.0
    WT_D[51, 2] = 1.0
    WT_D[52, 3] = 1.0
    WT_D[0:50, 6:56] = D4
    WT_D[53, 6:56] = dec4b2
    mk("WT_D", WT_D)

    import ml_dtypes
    for name in F32R_WEIGHTS:
        F[name] = F[name].astype(ml_dtypes.bfloat16)
    return F


# --------------------------------------------------------------------------
# bass kernel
# --------------------------------------------------------------------------
def build_kernel(rows, use_prelu=True):
    import concourse.bacc as bacc
    import concourse.tile as tile
    from concourse import mybir

    f32 = mybir.dt.float32
    f32r = mybir.dt.bfloat16
    nblk = rows // NB
    assert rows % NB == 0

    nc = bacc.Bacc("TRN2", target_bir_lowering=False, debug=False)

    x_d = nc.dram_tensor("x", [rows, 13], f32, kind="ExternalInput")
    ident_d = nc.dram_tensor("ident", [128, 128], f32, kind="ExternalInput")
    ones_d = nc.dram_tensor("ones", [1, NB], f32r, kind="ExternalInput")
    zeros_d = nc.dram_tensor("zeros", [50, NB], f32r, kind="ExternalInput")
    w_d = {
        name: nc.dram_tensor(
            name, list(shape), f32r if name in F32R_WEIGHTS else f32,
            kind="ExternalInput")
        for name, shape in WEIGHT_SHAPES.items()
    }
    out_d = nc.dram_tensor("out", [rows, 6], f32, kind="ExternalOutput")
    dec_d = nc.dram_tensor("decoded", [rows, 130], f32, kind="ExternalOutput")
    l1_d = nc.dram_tensor("l1in", [rows, 130], f32, kind="ExternalOutput")
    d4_d = nc.dram_tensor("dec4", [rows, 50], f32, kind="ExternalOutput")
    l4_d = nc.dram_tensor("l4in", [rows, 50], f32, kind="ExternalOutput")

    AF = mybir.ActivationFunctionType
    ALU = mybir.AluOpType

    with tile.TileContext(nc) as tc, ExitStack() as ctx:
        wpool = ctx.enter_context(tc.tile_pool(name="weights", bufs=1))
        sb = ctx.enter_context(tc.tile_pool(name="state", bufs=3))
        tout_pool = ctx.enter_context(tc.tile_pool(name="tout", bufs=2))
        xb_pool = ctx.enter_context(tc.tile_pool(name="xb", bufs=3))
        ps = ctx.enter_context(tc.tile_pool(name="ps", bufs=6, space="PSUM"))
        tps = ctx.enter_context(tc.tile_pool(name="tps", bufs=2, space="PSUM"))

        W = {}
        for name, shape in WEIGHT_SHAPES.items():
            t = wpool.tile(list(shape), f32r if name in F32R_WEIGHTS else f32,
                           tag=name)
            nc.sync.dma_start(t[:], w_d[name][:])
            W[name] = t
        ident = wpool.tile([128, 128], f32, tag="ident")
        nc.sync.dma_start(ident[:], ident_d[:])

        def act(out_ap, in_ap):
            if use_prelu:
                nc.scalar.activation(out_ap, in_ap, AF.Lrelu, bias=0.0,
                                     scale=1.0, alpha=0.01)
            else:
                nc.scalar.activation(out_ap, in_ap, AF.Relu, bias=0.0,
                                     scale=1.0)

        for b in range(nblk):
            r0 = b * NB
            # ---- load x block, transpose to feature-major -----------------
            xb = xb_pool.tile([128, 52], f32, tag="xb")
            nc.sync.dma_start(
                xb[:].rearrange("p (c f) -> p c f", c=4),
                x_d[r0:r0 + NB, :].rearrange("(p c) f -> p c f", c=4),
            )
            pX = ps.tile([13, NB], f32, tag="ps")
            for c in range(4):
                nc.tensor.transpose(
                    pX[0:13, c * 128:(c + 1) * 128],
                    xb[:, c * 13:(c + 1) * 13],
                    ident[:],
                )
            XT = sb.tile([66, NB], f32r, tag="XT")
            nc.vector.tensor_copy(XT[0:13, :], pX[0:13, :])
            nc.sync.dma_start(XT[13:14, :], ones_d[:])
            nc.sync.dma_start(XT[14:64, :], zeros_d[:])

            # ---- feature-major chain (float32r matmuls, N=512) ------------
            p1 = ps.tile([121, NB], f32, tag="ps")
            nc.tensor.matmul(p1[:], W["WZ1"][:], XT[0:14, :], start=True, stop=True)
            OUT1 = sb.tile([121, NB], f32r, tag="OUT1")
            act(OUT1[:], p1[:])

            pA = ps.tile([128, NB], f32, tag="ps")
            nc.tensor.matmul(pA[:], W["WD1A"][:], OUT1[:], start=True, stop=True)
            Y1A = sb.tile([128, NB], f32r, tag="Y1A")
            nc.vector.tensor_scalar(Y1A[:], pA[:], 0.0, None, ALU.max)

            pC = ps.tile([98, NB], f32, tag="ps")
            nc.tensor.matmul(pC[:], W["WO2AB"][:], OUT1[:], start=True, stop=False)
            nc.tensor.matmul(pC[:], W["WO2X"][:], XT[0:14, :], start=False, stop=True)
            OUT2 = sb.tile([71, NB], f32r, tag="OUT2")
            act(OUT2[:], pC[0:71, :])
            nc.vector.tensor_scalar(XT[64:66, :], pC[96:98, :], 0.0, None, ALU.max)

            pD = ps.tile([103, NB], f32, tag="ps")
            nc.tensor.matmul(pD[:], W["WO3"][:], OUT2[:], start=True, stop=True)
            OT3 = sb.tile([103, NB], f32r, tag="OT3")
            act(OT3[:], pD[:])

            pE = ps.tile([51, NB], f32, tag="ps")
            nc.tensor.matmul(pE[:], W["WO4O"][:], OT3[:], start=True, stop=False)
            nc.tensor.matmul(pE[:], W["WO4X"][:], XT[0:14, :], start=False, stop=True)
            OUT4 = sb.tile([51, NB], f32r, tag="OUT4")
            act(OUT4[:], pE[:])

            pF = ps.tile([54, NB], f32, tag="ps")
            nc.tensor.matmul(pF[:], W["WY4"][:], OUT4[:], start=True, stop=True)
            Y4H = sb.tile([54, NB], f32r, tag="Y4H")
            nc.vector.tensor_scalar(Y4H[:], pF[:], 0.0, None, ALU.max)

            pG = ps.tile([101, NB], f32, tag="ps")
            nc.tensor.matmul(pG[:], W["WO5"][:], OUT4[:], start=True, stop=True)
            OUT5 = sb.tile([101, NB], f32r, tag="OUT5")
            act(OUT5[:], pG[:])

            pH = ps.tile([2, NB], f32, tag="ps")
            nc.tensor.matmul(pH[:], W["WZ6O"][:], OUT5[:], start=True, stop=False)
            nc.tensor.matmul(pH[:], W["WZ6X"][:], XT[0:14, :], start=False, stop=True)
            act(XT[32:34, :], pH[:])

            # ---- terminal batch-major phase -------------------------------
            TOUT = tout_pool.tile([128, 4 * TP_W], f32, tag="TOUT")
            for c in range(4):
                cs = slice(c * 128, (c + 1) * 128)
                tp = tps.tile([128, TP_PAD], f32, tag="tp")
                nc.tensor.matmul(tp[:, 0:TP_PAD], XT[:, cs], W["WT_X"][:],
                                 start=True, stop=False)
                nc.tensor.matmul(tp[:, TC_DEC:TC_DEC + 256],
                                 Y1A[:, cs], W["WT_A"][:],
                                 start=False, stop=False)
                nc.tensor.matmul(tp[:, TC_L4:TC_L4 + 52],
                                 OT3[:, cs], W["WT_C"][:],
                                 start=False, stop=False)
                nc.tensor.matmul(tp[:, TC_OUT:TC_OUT + 56],
                                 Y4H[:, cs], W["WT_D"][:],
                                 start=False, stop=True)
                for i, (to, c0, w) in enumerate((
                    (TO_DEC, TC_DEC, 130),
                    (TO_L1, TC_L1, 130),
                    (TO_L4, TC_L4, 50),
                    (TO_D4, TC_D4, 50),
                    (TO_OUT, TC_OUT, 6),
                )):
                    dst = TOUT[:, to + c * w:to + (c + 1) * w]
                    if (c + i) % 2 == 0:
                        nc.vector.tensor_copy(dst, tp[:, c0:c0 + w])
                    else:
                        nc.scalar.copy(dst, tp[:, c0:c0 + w])

            # ---- output DMAs (4 rows per descriptor) ---------------------
            for dram, to, w in (
                (dec_d, TO_DEC, 130),
                (l1_d, TO_L1, 130),
                (l4_d, TO_L4, 50),
                (d4_d, TO_D4, 50),
                (out_d, TO_OUT, 6),
            ):
                nc.sync.dma_start(
                    dram[r0:r0 + NB, :].rearrange("(p c) f -> p (c f)", c=4),
                    TOUT[:, to:to + 4 * w],
                )

    nc.compile()
    return nc


_KERNEL_CACHE = {}


def _get_kernel(rows, use_prelu=True):
    key = (rows, use_prelu)
    if key not in _KERNEL_CACHE:
        _KERNEL_CACHE[key] = build_kernel(rows, use_prelu)
    return _KERNEL_CACHE[key]


def run_on_cores(x_full, folds, n_cores=N_CORES, trace=False, use_prelu=True):
    from concourse.bass_utils import run_bass_kernel_spmd

    x_full = np.ascontiguousarray(np.asarray(x_full, dtype=np.float32))
    total = x_full.shape[0]
    rows = total // n_cores
    assert rows * n_cores == total
    nc = _get_kernel(rows, use_prelu)

    base = {name: folds[name] for name in WEIGHT_SHAPES}
    base["ident"] = np.eye(128, dtype=np.float32)
    import ml_dtypes
    base["ones"] = np.ones((1, NB), dtype=ml_dtypes.bfloat16)
    base["zeros"] = np.zeros((50, NB), dtype=ml_dtypes.bfloat16)
    in_maps = []
    for i in range(n_cores):
        m = dict(base)
        m["x"] = x_full[i * rows:(i + 1) * rows]
        in_maps.append(m)

    res = run_bass_kernel_spmd(nc, in_maps, list(range(n_cores)), trace=trace)
    outs = []
    for name in ("out", "decoded", "l1in", "dec4", "l4in"):
        outs.append(np.concatenate([res.results[i][name] for i in range(n_cores)],
                                   axis=0))
    return outs, res


def kernel(**inputs):
    folds = build_folds(inputs)
    x = np.asarray(inputs["x"], dtype=np.float32)
    outs, _ = run_on_cores(x, folds)
    return tuple(outs)
